# revision 36
# baseline (speedup 1.0000x reference)
"""CoLA encoder layer on 8 Trainium2 NeuronCores, data-parallel over batch.

Key algebraic restructure vs the v1 kernel: Q = x@W_Q^T + b_Q is only ever
used for the logits S = Q@C_K, so the [D,D] Q GEMM folds into a host-side
precompute M = W_Q^T@C_K [D, ALPHA] (and cS = b_Q@C_K), turning the largest
on-chip matmul into an x^T @ M matmul with a 64-wide output. Per core (one
batch element, L=4096, D=1024, ALPHA=64):

  S = x@M (bf16), softmax+mask, attn = A@C_V^T (fp32r), LN1 residual,
  z^T via PE transpose (fp32r, 1.5 cyc/row), pointwise conv (fp32r),
  LeakyReLU, LN2 residual.  Elementwise work is spread over DVE, Act and
  GpSimd; emission order per chunk is [transposes+copies (c-1)] ->
  [frontend c] -> [conv backend (c-1)] so conv never waits on copies.
"""

import sys

for _p in ("/opt/trn_rl_repo",):
    if _p not in sys.path:
        sys.path.insert(0, _p)

from contextlib import ExitStack

import ml_dtypes
import numpy as np

import concourse.bass as bass
import concourse.tile as tile
from concourse import bacc, mybir
from concourse.bass_utils import run_bass_kernel_spmd

F32 = mybir.dt.float32
F32R = mybir.dt.float32r
BF16 = mybir.dt.bfloat16
AF = mybir.ActivationFunctionType
ALU = mybir.AluOpType
AX = mybir.AxisListType

B, L, D, ALPHA = 8, 4096, 1024, 64
EPS = 1e-6
INV_SQRT_D = 1.0 / 32.0
CH = 512           # tokens per chunk
NCH = L // CH      # 8 chunks
LTPC = CH // 128   # l-tiles per chunk
DT = D // 128      # feature tiles

_CACHE = {}


def _build(g1_trivial: bool, g2_trivial: bool, bq_trivial: bool = True,
           time_iters: int = 1):
    nc = bacc.Bacc("TRN2", target_bir_lowering=False, debug=False)

    x_d = nc.dram_tensor("x", [L, D], BF16, kind="ExternalInput")
    xt_d = nc.dram_tensor("xt", [D, L], BF16, kind="ExternalInput")
    m_d = nc.dram_tensor("m", [D, ALPHA], BF16, kind="ExternalInput")
    cv_d = nc.dram_tensor("cv", [ALPHA, D], BF16, kind="ExternalInput")
    wc_d = nc.dram_tensor("wc", [D, D], BF16, kind="ExternalInput")
    mask_d = nc.dram_tensor("maskf", [128, L // 128], F32, kind="ExternalInput")
    id_d = nc.dram_tensor("ident", [128, 128], BF16, kind="ExternalInput")
    bcr_d = nc.dram_tensor("bcr", [1, D], F32, kind="ExternalInput")
    if not bq_trivial:
        cs_d = nc.dram_tensor("cs", [1, ALPHA], BF16, kind="ExternalInput")
        ones_d = nc.dram_tensor("onesr", [1, 128], BF16, kind="ExternalInput")
    if not g1_trivial:
        g1_d = nc.dram_tensor("g1r", [1, D], F32, kind="ExternalInput")
        be1_d = nc.dram_tensor("be1r", [1, D], F32, kind="ExternalInput")
    if not g2_trivial:
        g2_d = nc.dram_tensor("g2r", [1, D], F32, kind="ExternalInput")
        be2_d = nc.dram_tensor("be2r", [1, D], F32, kind="ExternalInput")
    out_d = nc.dram_tensor("out", [L, D], BF16, kind="ExternalOutput")

    x_ap = x_d.ap()
    xt_ap = xt_d.ap().rearrange("(t p) l -> p t l", p=128)
    out_ap = out_d.ap()

    with tile.TileContext(nc) as tc:
        with ExitStack() as ctx:
            wp = ctx.enter_context(tc.tile_pool(name="weights", bufs=1))
            xt_pool = ctx.enter_context(tc.tile_pool(name="xtp", bufs=2))
            ht_pool = ctx.enter_context(tc.tile_pool(name="htp", bufs=2))
            xz_pool = ctx.enter_context(tc.tile_pool(name="xzp", bufs=8))
            zb_pool = ctx.enter_context(tc.tile_pool(name="zbp", bufs=12))
            ya_pool = ctx.enter_context(tc.tile_pool(name="yap", bufs=5))
            a_pool = ctx.enter_context(tc.tile_pool(name="ap", bufs=2))
            at_pool = ctx.enter_context(tc.tile_pool(name="atp", bufs=2))
            st_pool = ctx.enter_context(tc.tile_pool(name="stp", bufs=28))
            sq_pool = ctx.enter_context(tc.tile_pool(name="sqp", bufs=1))
            cb_pool = ctx.enter_context(tc.tile_pool(name="cbp", bufs=2))
            yb_pool = ctx.enter_context(tc.tile_pool(name="ybp", bufs=5))
            # PSUM banks: S 1 + A^T 1 + zT 2 + (attn|conv) 2x2 = 8
            ps_s = ctx.enter_context(tc.tile_pool(name="pss", bufs=1, space="PSUM"))
            ps_at = ctx.enter_context(tc.tile_pool(name="psat", bufs=1, space="PSUM"))
            ps_mm = ctx.enter_context(tc.tile_pool(name="psmm", bufs=2, space="PSUM"))
            ps_big = ctx.enter_context(tc.tile_pool(name="psbig", bufs=2, space="PSUM"))

            m_sb = wp.tile([128, DT, ALPHA], BF16)
            nc.sync.dma_start(m_sb, m_d.ap().rearrange("(t p) a -> p t a", p=128))
            cv_sb = wp.tile([ALPHA, D], BF16)
            nc.sync.dma_start(cv_sb, cv_d.ap())
            id_sb = wp.tile([128, 128], BF16)
            nc.sync.dma_start(id_sb, id_d.ap())
            bc_sb = wp.tile([128, D], F32)
            nc.sync.dma_start(bc_sb, bcr_d.ap().to_broadcast((128, D)))
            mask_sb = wp.tile([128, L // 128], F32)
            nc.sync.dma_start(mask_sb, mask_d.ap())
            if not bq_trivial:
                cs_sb = wp.tile([1, ALPHA], BF16)
                nc.sync.dma_start(cs_sb, cs_d.ap())
                ones_sb = wp.tile([1, 128], BF16)
                nc.sync.dma_start(ones_sb, ones_d.ap())
            if not g1_trivial:
                g1_sb = wp.tile([128, D], F32)
                nc.sync.dma_start(g1_sb, g1_d.ap().to_broadcast((128, D)))
                be1_sb = wp.tile([128, D], F32)
                nc.sync.dma_start(be1_sb, be1_d.ap().to_broadcast((128, D)))
            if not g2_trivial:
                g2_sb = wp.tile([128, D], F32)
                nc.sync.dma_start(g2_sb, g2_d.ap().to_broadcast((128, D)))
                be2_sb = wp.tile([128, D], F32)
                nc.sync.dma_start(be2_sb, be2_d.ap().to_broadcast((128, D)))
            # big conv weight last so it doesn't block the x/xt stream;
            # issue from the Act queue to overlap with SP-queue input DMA
            wc_sb = wp.tile([128, DT, D], BF16)
            nc.scalar.dma_start(wc_sb, wc_d.ap().rearrange("(t p) e -> p t e", p=128))

            def frontend(c):
                """DMA, S=x@M, softmax, attn, residual, LN1 -> z in xz."""
                l0 = c * CH
                xt_sb = xt_pool.tile([128, DT, CH], BF16, tag="xt")
                nc.sync.dma_start(xt_sb, xt_ap[:, :, l0 : l0 + CH])
                xz = []
                for lt in range(LTPC):
                    t = xz_pool.tile([128, D], BF16, name=f"xz{lt}", tag="xz")
                    nc.sync.dma_start(
                        t, x_ap[l0 + lt * 128 : l0 + (lt + 1) * 128, :]
                    )
                    xz.append(t)

                # logits for all 4 l-tiles into one PSUM bank [128, 4, 64]
                ps = ps_s.tile([128, LTPC, ALPHA], F32, name="ps", tag="ps")
                for lt in range(LTPC):
                    l1 = lt * 128
                    for d in range(DT):
                        nc.tensor.matmul(
                            ps[:, lt, :],
                            xt_sb[:, d, l1 : l1 + 128],
                            m_sb[:, d, :],
                            start=(d == 0),
                            stop=(d == DT - 1 and bq_trivial),
                        )
                    if not bq_trivial:
                        nc.tensor.matmul(
                            ps[:, lt, :], ones_sb, cs_sb, start=False, stop=True
                        )
                # softmax, batched over the chunk; logits are tiny (|S|/32
                # < 0.25) so the max-subtraction is unnecessary
                ev4 = a_pool.tile([128, LTPC, ALPHA], BF16, name="ev4", tag="ev4")
                nc.scalar.activation(ev4, ps, AF.Exp, scale=INV_SQRT_D)
                sm4 = st_pool.tile([128, LTPC], F32, name="sm4", tag="sm4")
                nc.vector.reduce_sum(sm4, ev4, axis=AX.X)
                rc4 = st_pool.tile([128, LTPC], F32, name="rc4", tag="rc4")
                nc.vector.reciprocal(rc4, sm4)
                r24 = st_pool.tile([128, LTPC], F32, name="r24", tag="r24")
                nc.vector.tensor_mul(
                    r24, rc4, mask_sb[:, c * LTPC : (c + 1) * LTPC]
                )
                r2b = bass.AP(
                    tensor=r24.tensor,
                    offset=r24.offset,
                    ap=[r24.ap[0], r24.ap[1], [0, ALPHA]],
                )
                nc.vector.tensor_mul(ev4, ev4, r2b)

                # A^T for the whole chunk in one PSUM bank, one copy out
                pat = ps_at.tile([ALPHA, CH], BF16, name="pat", tag="pat")
                for lt in range(LTPC):
                    nc.tensor.transpose(
                        pat[:, lt * 128 : (lt + 1) * 128], ev4[:, lt, :], id_sb
                    )
                at_sb = at_pool.tile([ALPHA, CH], BF16, name="at", tag="at")
                nc.vector.tensor_copy(at_sb, pat)

                sum1 = st_pool.tile([128, LTPC], F32, name="sum1", tag="sum1")
                ssq1 = st_pool.tile([128, LTPC], F32, name="ssq1", tag="ssq1")
                for lt in range(LTPC):
                    # attn + residual; accum_out gives sum(r) for LN1
                    pa = ps_big.tile([128, D], F32, name="pa", tag="mm1024")
                    for hf in range(2):
                        nc.tensor.matmul(
                            pa[:, hf * 512 : (hf + 1) * 512],
                            at_sb[:, lt * 128 : (lt + 1) * 128],
                            cv_sb[:, hf * 512 : (hf + 1) * 512],
                            start=True,
                            stop=True,
                        )
                    nc.vector.scalar_tensor_tensor(
                        xz[lt], xz[lt], 0.0, pa, ALU.add, ALU.add,
                        accum_out=sum1[:, lt : lt + 1],
                    )
                    sq = sq_pool.tile([128, D], F32, name="sq", tag="sq")
                    nc.scalar.activation(
                        sq, xz[lt], AF.Square, accum_out=ssq1[:, lt : lt + 1]
                    )

                # LN1 stats, batched [128, 4]: mean = sum/D,
                # var = (ssq - D*mean^2)/(D-1), inv = 1/(sqrt(var)+eps)
                mean1 = st_pool.tile([128, LTPC], F32, name="mean1", tag="mean1")
                nc.vector.tensor_scalar_mul(mean1, sum1, 1.0 / D)
                var1 = st_pool.tile([128, LTPC], F32, name="var1", tag="var1")
                nc.vector.tensor_mul(var1, mean1, mean1)
                nc.vector.scalar_tensor_tensor(
                    var1, var1, -float(D), ssq1, ALU.mult, ALU.add
                )
                sd1 = st_pool.tile([128, LTPC], F32, name="sd1", tag="sd1")
                nc.scalar.activation(sd1, var1, AF.Sqrt, scale=1.0 / (D - 1))
                nc.vector.tensor_scalar_add(sd1, sd1, EPS)
                iv1 = st_pool.tile([128, LTPC], F32, name="iv1", tag="iv1")
                nc.vector.reciprocal(iv1, sd1)
                # nmi = -mean*iv so LN1-normalize runs on Act as
                # Identity(xz*iv + nmi) with per-partition scale/bias APs
                nmi1 = st_pool.tile([128, LTPC], F32, name="nmi1", tag="nmi1")
                nc.vector.scalar_tensor_tensor(
                    nmi1, mean1, -1.0, iv1, ALU.mult, ALU.mult
                )
                zb = []
                for lt in range(LTPC):
                    z = zb_pool.tile([128, D], BF16, name=f"zb{lt}", tag="zb")
                    nc.vector.tensor_scalar(
                        z, xz[lt], mean1[:, lt : lt + 1],
                        iv1[:, lt : lt + 1], ALU.subtract, ALU.mult,
                    )
                    if not g1_trivial:
                        nc.vector.tensor_mul(z, z, g1_sb)
                        nc.vector.tensor_add(z, z, be1_sb)
                    zb.append(z)
                return zb

            def backend_pre(c, zb):
                """z^T via PE transpose (bf16, 1 cyc/row), PSUM->SBUF copies."""
                ht_sb = ht_pool.tile([128, DT, CH], BF16, tag="ht")
                for d in range(DT):
                    pzt = ps_mm.tile([128, CH], BF16, name="pzt", tag="mm512")
                    for lt in range(LTPC):
                        nc.tensor.transpose(
                            pzt[:, lt * 128 : (lt + 1) * 128],
                            zb[lt][:, d * 128 : (d + 1) * 128],
                            id_sb,
                        )
                    if d % 2 == 0:
                        nc.scalar.activation(ht_sb[:, d, :], pzt, AF.Copy)
                    else:
                        nc.vector.tensor_copy(ht_sb[:, d, :], pzt)
                return ht_sb

            def backend_post(c, zb, ht_sb):
                """conv, bias, LeakyReLU, residual, LN2, DMA out."""
                l0 = c * CH
                sum2 = st_pool.tile([128, LTPC], F32, name="sum2", tag="sum2")
                ssq2 = st_pool.tile([128, LTPC], F32, name="ssq2", tag="ssq2")
                yas = []
                for lt in range(LTPC):
                    l1 = lt * 128
                    pc = ps_big.tile([128, D], F32, name="pc", tag="mm1024")
                    for hf in range(2):
                        pch = pc[:, hf * 512 : (hf + 1) * 512]
                        for d in range(DT):
                            nc.tensor.matmul(
                                pch,
                                ht_sb[:, d, l1 : l1 + 128],
                                wc_sb[:, d, hf * 512 : (hf + 1) * 512],
                                start=(d == 0),
                                stop=(d == DT - 1),
                            )
                    # bias add on DVE, then leaky on Act
                    cb = cb_pool.tile([128, D], F32, name="cb", tag="cb")
                    nc.vector.tensor_add(cb, pc, bc_sb)
                    nc.scalar.activation(cb, cb, AF.Lrelu, alpha=0.01)
                    ya = ya_pool.tile([128, D], F32, name="ya", tag="ya")
                    yas.append(ya)
                    nc.vector.scalar_tensor_tensor(
                        ya, cb, 0.0, zb[lt], ALU.add, ALU.add,
                        accum_out=sum2[:, lt : lt + 1],
                    )
                    sq2 = sq_pool.tile([128, D], F32, name="sq2", tag="sq")
                    nc.scalar.activation(
                        sq2, ya, AF.Square, accum_out=ssq2[:, lt : lt + 1]
                    )

                mean2 = st_pool.tile([128, LTPC], F32, name="mean2", tag="mean2")
                nc.vector.tensor_scalar_mul(mean2, sum2, 1.0 / D)
                var2 = st_pool.tile([128, LTPC], F32, name="var2", tag="var2")
                nc.vector.tensor_mul(var2, mean2, mean2)
                nc.vector.scalar_tensor_tensor(
                    var2, var2, -float(D), ssq2, ALU.mult, ALU.add
                )
                sd2 = st_pool.tile([128, LTPC], F32, name="sd2", tag="sd2")
                nc.scalar.activation(sd2, var2, AF.Sqrt, scale=1.0 / (D - 1))
                nc.vector.tensor_scalar_add(sd2, sd2, EPS)
                iv2 = st_pool.tile([128, LTPC], F32, name="iv2", tag="iv2")
                nc.vector.reciprocal(iv2, sd2)
                for lt in range(LTPC):
                    ya = yas[lt]
                    yb = yb_pool.tile([128, D], BF16, name="yb", tag="yb")
                    nc.vector.tensor_scalar(
                        yb, ya, mean2[:, lt : lt + 1], iv2[:, lt : lt + 1],
                        ALU.subtract, ALU.mult,
                    )
                    if not g2_trivial:
                        nc.vector.tensor_mul(yb, yb, g2_sb)
                        nc.vector.tensor_add(yb, yb, be2_sb)
                    ya = yb
                    if time_iters > 1:
                        # accumulate so unrolled timing passes stay live
                        # (defeats dead-code elimination); SWDGE required
                        nc.gpsimd.dma_start(
                            out_ap[l0 + lt * 128 : l0 + (lt + 1) * 128, :],
                            ya, accum_op=ALU.add,
                        )
                    else:
                        nc.sync.dma_start(
                            out_ap[l0 + lt * 128 : l0 + (lt + 1) * 128, :], ya
                        )

            # 3-deep software pipeline: per iteration emit
            #   backend_pre(c-1): z^T transposes + PSUM copies
            #   frontend(c):      logits/softmax/attn/LN1
            #   backend_post(c-2): conv (reads ht copied a full iteration
            #                      earlier, so it never waits on copies)
            # time_iters>1 re-emits the whole body (straight-line unroll)
            # for steady-state timing.
            stages = []  # (c, xz) awaiting backend_pre
            ready = []   # (c, xz, ht) awaiting backend_post
            for _it in range(time_iters):
                for c in range(NCH):
                    if stages:
                        pc_, pxz = stages.pop(0)
                        ready.append((pc_, pxz, backend_pre(pc_, pxz)))
                    xz_c = frontend(c)
                    if len(ready) > 1:
                        bc_, bxz, bht = ready.pop(0)
                        backend_post(bc_, bxz, bht)
                    stages.append((c, xz_c))
            while stages:
                pc_, pxz = stages.pop(0)
                ready.append((pc_, pxz, backend_pre(pc_, pxz)))
            for bc_, bxz, bht in ready:
                backend_post(bc_, bxz, bht)


    nc.compile()
    return nc


def _get_nc(g1_trivial, g2_trivial, bq_trivial):
    key = (g1_trivial, g2_trivial, bq_trivial)
    if key not in _CACHE:
        _CACHE[key] = _build(*key)
    return _CACHE[key]


def build_in_maps(x, mask, W_Q, b_Q, C_K, C_V, g1, be1, Wc, bc, g2, be2):
    """Host-side prep shared by kernel() and test harnesses.

    Returns (nc, in_maps)."""
    x = np.asarray(x, dtype=np.float32)
    mask = np.asarray(mask)
    W_Q = np.asarray(W_Q, dtype=np.float32)
    b_Q = np.asarray(b_Q, dtype=np.float32)
    C_K = np.asarray(C_K, dtype=np.float32)
    C_V = np.asarray(C_V, dtype=np.float32)
    g1 = np.asarray(g1, dtype=np.float32)
    be1 = np.asarray(be1, dtype=np.float32)
    Wc = np.asarray(Wc, dtype=np.float32)
    bc = np.asarray(bc, dtype=np.float32)
    g2 = np.asarray(g2, dtype=np.float32)
    be2 = np.asarray(be2, dtype=np.float32)

    g1_trivial = bool(np.all(g1 == 1.0) and np.all(be1 == 0.0))
    g2_trivial = bool(np.all(g2 == 1.0) and np.all(be2 == 0.0))
    bq_trivial = bool(np.all(b_Q == 0.0))
    nc = _get_nc(g1_trivial, g2_trivial, bq_trivial)

    # Q only feeds the logits, so W_Q/C_K collapse on the host
    m16 = np.ascontiguousarray(W_Q.T @ C_K).astype(ml_dtypes.bfloat16)
    wcT = np.ascontiguousarray(Wc.T).astype(ml_dtypes.bfloat16)
    cvT = np.ascontiguousarray(C_V.T).astype(ml_dtypes.bfloat16)
    bc_row = bc.reshape(1, D)
    ident = np.eye(128, dtype=np.float32)

    in_maps = []
    for b in range(B):
        m = {
            "x": np.ascontiguousarray(x[b].astype(ml_dtypes.bfloat16)),
            "xt": np.ascontiguousarray(x[b].T.astype(ml_dtypes.bfloat16)),
            "m": m16,
            "wc": wcT,
            "cv": cvT,
            "bcr": bc_row,
            "maskf": np.ascontiguousarray(
                mask[b].astype(np.float32).reshape(L // 128, 128).T
            ),
            "ident": ident.astype(ml_dtypes.bfloat16),
        }
        if not bq_trivial:
            m["cs"] = (b_Q @ C_K).reshape(1, ALPHA).astype(ml_dtypes.bfloat16)
            m["onesr"] = np.ones((1, 128), dtype=ml_dtypes.bfloat16)
        if not g1_trivial:
            m["g1r"] = g1.reshape(1, D)
            m["be1r"] = be1.reshape(1, D)
        if not g2_trivial:
            m["g2r"] = g2.reshape(1, D)
            m["be2r"] = be2.reshape(1, D)
        in_maps.append(m)
    return nc, in_maps


def kernel(x, mask, W_Q, b_Q, C_K, C_V, g1, be1, Wc, bc, g2, be2):
    nc, in_maps = build_in_maps(
        x, mask, W_Q, b_Q, C_K, C_V, g1, be1, Wc, bc, g2, be2
    )
    res = run_bass_kernel_spmd(nc, in_maps, core_ids=list(range(B)))
    return np.stack(
        [np.asarray(res.results[b]["out"]) for b in range(B)], axis=0
    ).astype(np.float32)
